# revision 41
# baseline (speedup 1.0000x reference)
"""Trainium2 Bass kernel for a dense transformer block (B=2, T=2048, C=1024, 16 heads).

Strategy (zero-collective, uniform SPMD over 8 cores):
  - cores 0-3 handle batch 0, cores 4-7 batch 1 (4 cores per sequence)
  - core with j = core%4 owns the stride-4 interleaved tokens {4u+j}: its
    q-tile qi (128 tokens covering positions [512*qi, 512*qi+512)) attends
    exactly k-tiles [0, 4*(qi+1)) with only the last 4 masked, so every
    core runs an identical, exactly-causal instruction stream (4+8+12+16
    = 40 k-tile units vs 34 true causal); masks are per-core data.
  - every core recomputes LN1 + k/v for its full 2048-token sequence,
    which removes all cross-core communication (collectives measured
    ~430-520us per op here -- far too slow to shard k/v).
  - LayerNorm subtracts the mean directly (DVE/Pool + gpsimd
    partition_broadcast; no rank-1 correction matmuls); the v bias is
    folded into the proj bias on the host (softmax rows sum to 1).
  - half-1 k/v GEMM emission is interleaved with attention part 0 so the
    in-order PE queue has matmul work while ACT runs the softmax exps.
  - attention (q/k/v/exp), proj and mlp2 run bf16; LN stats and mlp1 use
    float32r on f32 data (full-rate on the PE at N>=256).

kernel(**inputs) takes the full unsharded inputs and returns the full
[2, 2048, 1024] output.
"""
import numpy as np
import ml_dtypes

import concourse.bass as bass
import concourse.tile as tile
from concourse import bacc, mybir
from concourse.bass_utils import run_bass_kernel_spmd

BF16 = ml_dtypes.bfloat16
F32 = mybir.dt.float32
F32R = mybir.dt.float32r
DBF = mybir.dt.bfloat16

C = 1024          # embed dim
T = 2048          # seq len
B = 2
H = 16            # heads
D = 64            # head dim
HID = 4096
NC = 8            # cores
CH = C // 128     # 8 channel chunks
QT = 128          # query tile width (stride-4 interleaved tokens)
LN_EPS = 1e-5
ATT_SCALE = 1.0 / 8.0   # 1/sqrt(64)

_BUILD_CACHE = {}


def r32(ap):
    """View an f32 access pattern as float32r for full-rate matmuls."""
    return ap.bitcast(F32R)


def f32(ap):
    """View a float32r access pattern as plain f32 for vector-engine ops."""
    return ap.bitcast(F32)


def build_kernel(reps=1, skip=()):
    key = (reps, tuple(skip))
    if key in _BUILD_CACHE:
        return _BUILD_CACHE[key]
    nc = bacc.Bacc("TRN2", target_bir_lowering=False, debug=False, num_devices=NC)

    # ---- I/O ----
    x_ctx_t = nc.dram_tensor("x_ctx", [CH, 128, T], F32R, kind="ExternalInput")
    x_own_t = nc.dram_tensor("x_own", [CH, 128, 512], F32R, kind="ExternalInput")
    wq_t = nc.dram_tensor("wq", [8, 128, CH, 128], DBF, kind="ExternalInput")
    wk_t = nc.dram_tensor("wk", [8, 128, CH, 128], DBF, kind="ExternalInput")
    wv_t = nc.dram_tensor("wv", [CH, 128, C], DBF, kind="ExternalInput")
    wp_t = nc.dram_tensor("wp", [8, 128, CH, 128], DBF, kind="ExternalInput")
    w1_t = nc.dram_tensor("w1", [32, 128, CH, 128], DBF, kind="ExternalInput")
    w2_t = nc.dram_tensor("w2", [8, 128, 32, 128], DBF, kind="ExternalInput")
    bq_t = nc.dram_tensor("bq", [128, 8], F32, kind="ExternalInput")
    bk_t = nc.dram_tensor("bk", [128, 8], F32, kind="ExternalInput")
    bp_t = nc.dram_tensor("bp", [128, 8], F32, kind="ExternalInput")
    b1_t = nc.dram_tensor("b1", [128, 32], F32, kind="ExternalInput")
    b2_t = nc.dram_tensor("b2", [128, 8], F32, kind="ExternalInput")
    mask_t = nc.dram_tensor("mask", [128, 16, QT], DBF, kind="ExternalInput")
    out_t = nc.dram_tensor("out", [CH, 128, 512], F32, kind="ExternalOutput")

    with tile.TileContext(nc) as tc, nc.allow_low_precision(
            reason="float32r operands feeding full-rate matmuls"):
        def body(it):
            with (
                tc.tile_pool(name=f"const{it}", bufs=1) as const,
                tc.tile_pool(name=f"bigp{it}", bufs=1) as bigp,
                tc.tile_pool(name=f"wpool{it}", bufs=2) as wpool,
                tc.tile_pool(name=f"stat{it}", bufs=1) as stat,
                tc.tile_pool(name=f"tmp{it}", bufs=2) as tmp,
            ):
                ones_f = const.tile([128, 128], F32)
                nc.vector.memset(ones_f, 1.0)
                ones_col = const.tile([128, 1], F32R)
                nc.vector.tensor_copy(ones_col, ones_f[:, 0:1])
                ones_row = const.tile([1, 128], F32R)
                nc.vector.tensor_copy(ones_row, ones_f[0:1, :])
                eps_c = const.tile([1, 1], F32)
                nc.vector.memset(eps_c, LN_EPS)

                def cload(name, t, shape, dtype=F32):
                    s = const.tile(shape, dtype, name=name)
                    nc.sync.dma_start(out=s, in_=t[tuple(slice(None) for _ in shape)])
                    return s

                x_own_sb = bigp.tile([128, CH, 512], F32R, tag="xown")
                for ci in range(CH):
                    nc.sync.dma_start(out=x_own_sb[:, ci, :], in_=x_own_t[ci, :, :])

                bq_sb = cload("bqs", bq_t, [128, 8])
                bk_sb = cload("bks", bk_t, [128, 8])
                bp_sb = cload("bps", bp_t, [128, 8])
                b1_sb = cload("b1s", b1_t, [128, 32])
                b2_sb = cload("b2s", b2_t, [128, 8])
                mask_sb = cload("masks", mask_t, [128, 16, QT], DBF)

                def ln_group(x_ap, w, out_ap_fn, ps_ln, gi, sq_dve=False):
                    """LayerNorm of one <=512-token group: writes
                    out = (x - mu) * rstd (per token).  LN gain/bias are
                    folded into the following weights/biases on the host."""
                    ps_s = ps_ln.tile([1, 512], F32, tag="s", bufs=2, name=f"pss_{it}_{gi}")
                    ps_q = ps_ln.tile([1, 512], F32, tag="q", bufs=2, name=f"psq_{it}_{gi}")
                    for ci in range(CH):
                        nc.tensor.matmul(
                            ps_s[:, :w], ones_col, x_ap[:, ci, :],
                            start=(ci == 0), stop=(ci == CH - 1))
                    for ci in range(CH):
                        sq = tmp.tile([128, 512], F32R, tag="sq", bufs=1,
                                      name=f"sq_{it}_{gi}_{ci}")
                        sq_eng = nc.vector if sq_dve else nc.gpsimd
                        sq_eng.tensor_mul(sq[:, :w], f32(x_ap[:, ci, :]),
                                          f32(x_ap[:, ci, :]))
                        nc.tensor.matmul(
                            ps_q[:, :w], ones_col, sq[:, :w],
                            start=(ci == 0), stop=(ci == CH - 1))
                    mu = stat.tile([1, 512], F32, tag="mu", name=f"mu_{it}_{gi}")
                    nc.vector.tensor_scalar_mul(mu[:, :w], ps_s[:, :w], 1.0 / C)
                    ex2 = stat.tile([1, 512], F32, tag="ex2", name=f"ex2_{it}_{gi}")
                    nc.vector.tensor_scalar_mul(ex2[:, :w], ps_q[:, :w], 1.0 / C)
                    var = stat.tile([1, 512], F32, tag="var", name=f"var_{it}_{gi}")
                    nc.scalar.activation(var[:, :w], mu[:, :w],
                                         mybir.ActivationFunctionType.Square)
                    nc.vector.tensor_sub(var[:, :w], ex2[:, :w], var[:, :w])
                    nc.scalar.activation(var[:, :w], var[:, :w],
                                         mybir.ActivationFunctionType.Sqrt,
                                         bias=eps_c[:, :])
                    rstd = stat.tile([1, 512], F32, tag="ex2", name=f"rstd_{it}_{gi}")
                    nc.vector.reciprocal(rstd[:, :w], var[:, :w])
                    mub = tmp.tile([128, 512], F32, tag="mub", bufs=2,
                                   name=f"mub_{it}_{gi}")
                    nc.gpsimd.partition_broadcast(mub[:, :w], mu[:, :w])
                    rsb = tmp.tile([128, 512], F32, tag="rsb", bufs=2,
                                   name=f"rsb_{it}_{gi}")
                    nc.gpsimd.partition_broadcast(rsb[:, :w], rstd[:, :w])
                    for ci in range(CH):
                        xc = tmp.tile([128, 512], F32, tag="xc", bufs=2,
                                      name=f"xc_{it}_{gi}_{ci}")
                        nc.gpsimd.tensor_sub(xc[:, :w], f32(x_ap[:, ci, :]),
                                             mub[:, :w])
                        nc.vector.tensor_mul(out_ap_fn(ci), xc[:, :w],
                                             rsb[:, :w])

                h1o = bigp.tile([128, CH, 512], DBF, tag="h1o")
                q_sb = bigp.tile([128, 8, 512], DBF, tag="qsb")
                k_sb = bigp.tile([128, 8, T], DBF, tag="ksb")
                v_sb = bigp.tile([128, 16, H, D + 1], DBF, tag="vsb")
                nc.vector.memset(v_sb[:, :, :, D:D + 1], 1.0)
                if 'kv' in skip:
                    nc.vector.memset(k_sb, 0.01)
                    nc.vector.memset(v_sb[:, :, :, 0:D], 0.01)

                y_sb = bigp.tile([128, 8, 512], DBF, tag="h1o")

                with tc.tile_pool(name=f"psacc{it}", bufs=2, space="PSUM") as ps_acc:
                    wv_sb = bigp.tile([128, CH, C], DBF, tag="wv")
                    h1hs = {}

                    def k_gemm(half, mt):
                        t0 = 1024 * half
                        h1h = h1hs[half]
                        wk_sb = wpool.tile([128, CH, 128], DBF, tag="w",
                                           name=f"wk_{it}_{half}_{mt}")
                        nc.sync.dma_start(out=wk_sb, in_=wk_t[mt, :, :, :])
                        for g in range(2):
                            g0 = t0 + 512 * g
                            ps = ps_acc.tile([128, 512], F32, tag="acc",
                                             name=f"psk_{it}_{half}_{mt}_{g}")
                            for ci in range(CH):
                                nc.tensor.matmul(
                                    ps, wk_sb[:, ci, :],
                                    h1h[:, ci, 512 * g:512 * g + 512],
                                    start=(ci == 0), stop=(ci == CH - 1))
                            nc.vector.tensor_scalar_add(
                                k_sb[:, mt, g0:g0 + 512], ps,
                                bk_sb[:, mt:mt + 1])

                    def v_gemm(half, tt):
                        # v bias is folded into the proj bias on the host
                        # (softmax rows sum to one, so y = AV/den + bv exactly)
                        h1h = h1hs[half]
                        gtt = 8 * half + tt
                        for vh in range(2):
                            ps = ps_acc.tile([128, 512], F32, tag="acc",
                                             name=f"psv_{it}_{half}_{tt}_{vh}")
                            for ci in range(CH):
                                nc.tensor.matmul(
                                    ps, h1h[:, ci, 128 * tt:128 * tt + 128],
                                    wv_sb[:, ci, 512 * vh:512 * vh + 512],
                                    start=(ci == 0), stop=(ci == CH - 1))
                            nc.vector.tensor_copy(
                                v_sb[:, gtt, 8 * vh:8 * vh + 8, 0:D],
                                ps.rearrange("p (h d) -> p h d", h=8))

                    with tc.tile_pool(name=f"psln{it}", bufs=1,
                                      space="PSUM") as ps_ln:
                        # ---- LN1 over own tokens -> h1o, then q ----
                        # (ctx group c00's stats are emitted between the two
                        # so PE has matmul work during the own-normalize tail)
                        ln_group(x_own_sb, 512,
                                 lambda ci: h1o[:, ci, :], ps_ln, "own",
                                 sq_dve=True)
                        h1hs[0] = bigp.tile([128, CH, 1024], DBF,
                                            tag="h1h0", name=f"h1h_{it}_0")
                        xg0 = bigp.tile([128, CH, 512], F32R, tag="xctx",
                                        name=f"xg_{it}_0_0")
                        for ci in range(CH):
                            nc.sync.dma_start(out=xg0[:, ci, :],
                                              in_=x_ctx_t[ci, :, 0:512])
                        ln_group(xg0, 512,
                                 lambda ci: h1hs[0][:, ci, 0:512],
                                 ps_ln, "c00")
                        for mt in range(8):
                            wq_sb = wpool.tile([128, CH, 128], DBF, tag="w",
                                               name=f"wq_{it}_{mt}")
                            nc.sync.dma_start(out=wq_sb, in_=wq_t[mt, :, :, :])
                            ps = ps_acc.tile([128, 512], F32, tag="acc",
                                             name=f"psq2_{it}_{mt}")
                            for ci in range(CH):
                                nc.tensor.matmul(ps, wq_sb[:, ci, :],
                                                 h1o[:, ci, :],
                                                 start=(ci == 0),
                                                 stop=(ci == CH - 1))
                            nc.vector.tensor_scalar_add(
                                q_sb[:, mt, :], ps, bq_sb[:, mt:mt + 1])

                        for ci in range(CH):
                            nc.sync.dma_start(out=wv_sb[:, ci, :],
                                              in_=wv_t[ci, :, :])

                        # ---- LN1 of remaining ctx groups, all before the
                        # half-0 k/v GEMMs: the GEMMs then provide PE filler
                        # while the later groups' normalize chains run, and
                        # the half-1 x_ctx stages load early enough for the
                        # interleaved half-1 k/v during attention part 0 ----
                        h1hs[1] = bigp.tile([128, CH, 1024], DBF, tag="h1h1",
                                            name=f"h1h_{it}_1")
                        for half, g in ((0, 1), (1, 0), (1, 1)):
                            g0 = 1024 * half + 512 * g
                            xg = bigp.tile([128, CH, 512], F32R, tag="xctx",
                                           name=f"xg_{it}_{half}_{g}")
                            for ci in range(CH):
                                nc.sync.dma_start(
                                    out=xg[:, ci, :],
                                    in_=x_ctx_t[ci, :, g0:g0 + 512])
                            ln_group(
                                xg, 512,
                                lambda ci, half=half, g=g: h1hs[half][
                                    :, ci, 512 * g:512 * g + 512],
                                ps_ln, f"c{half}{g}")
                        if 'kv' not in skip:
                            for mt in range(8):
                                k_gemm(0, mt)
                            for tt in range(8):
                                v_gemm(0, tt)

                    # ---------- attention (stride-4 interleaved queries),
                    # half-1 k/v GEMMs emission-interleaved with part 0 so PE
                    # has matmul work while ACT runs softmax exps ----------
                    # q-tile qi = positions {512*qi + 4u + j}; it attends
                    # exactly k-tiles [0, 4*(qi+1)), the last 4 masked.
                    with (
                        tc.tile_pool(name=f"psscr{it}", bufs=2,
                                     space="PSUM") as ps_scr,
                        tc.tile_pool(name=f"psy{it}", bufs=1,
                                     space="PSUM") as ps_y,
                        tc.tile_pool(name=f"esp{it}", bufs=2) as espool,
                    ):
                        def att_part(rt, part):
                            hh = [2 * rt, 2 * rt + 1]
                            psYs = [ps_y.tile([D + 1, 256], F32, tag=f"y{u}",
                                              name=f"psY_{it}_{rt}_{part}_{u}")
                                    for u in range(2)]
                            for qq_ in range(2):
                                qi = 2 * part + qq_
                                q0 = QT * qi
                                nkt = 4 * (qi + 1)
                                ess = [espool.tile([128, 16, QT], DBF,
                                                   tag=f"es{u}",
                                                   name=f"es_{it}_{rt}_{qi}_{u}")
                                       for u in range(2)]
                                for kb in range(qi + 1):
                                    pss = [ps_scr.tile(
                                        [128, 4, QT], F32, tag=f"scr{u}",
                                        name=f"psS_{it}_{rt}_{qi}_{kb}_{u}")
                                        for u in range(2)]
                                    for i in range(4):
                                        kt = 4 * kb + i
                                        for u in range(2):
                                            po = 64 * u
                                            nc.tensor.matmul(
                                                pss[u][:, i, :],
                                                k_sb[po:po + 64, rt,
                                                     128 * kt:128 * kt + 128],
                                                q_sb[po:po + 64, rt,
                                                     q0:q0 + QT],
                                                start=True, stop=True)
                                    for u in range(2):
                                        nc.scalar.activation(
                                            ess[u][:, 4 * kb:4 * kb + 4, :],
                                            pss[u],
                                            mybir.ActivationFunctionType.Exp,
                                            scale=ATT_SCALE)
                                        if kb == qi:   # boundary k-tiles
                                            nc.vector.tensor_mul(
                                                ess[u][:, 4 * kb:4 * kb + 4, :],
                                                ess[u][:, 4 * kb:4 * kb + 4, :],
                                                mask_sb[:, 4 * qi:4 * qi + 4, :])
                                    for u in range(2):
                                        for i in range(4):
                                            kt = 4 * kb + i
                                            nc.tensor.matmul(
                                                psYs[u][:, 128 * qq_:
                                                        128 * qq_ + 128],
                                                v_sb[:, kt, hh[u], :],
                                                ess[u][:, kt, :],
                                                start=(kt == 0),
                                                stop=(kt == nkt - 1),
                                                skip_group_check=True)
                            for u in range(2):
                                po = 64 * u
                                psY = psYs[u]
                                rd = stat.tile([1, 256], F32, tag="mu",
                                               name=f"rd_{it}_{rt}_{part}_{u}")
                                nc.vector.reciprocal(rd, psY[D:D + 1, :])
                                rdb = tmp.tile([64, 256], F32, tag="rdb",
                                               bufs=2,
                                               name=f"rdb_{it}_{rt}_{part}_{u}")
                                nc.gpsimd.partition_broadcast(rdb, rd)
                                nc.vector.tensor_mul(
                                    y_sb[po:po + 64, rt,
                                         256 * part:256 * part + 256],
                                    psY[0:D, :], rdb)

                        att_rts = [] if 'att' in skip else list(range(8))
                        if 'att' in skip:
                            nc.vector.tensor_copy(y_sb.rearrange('p a b -> p (a b)'), ones_f[:, 0:1].broadcast_to((128, CH * 512)))
                        # part 0 needs only half-0 k/v; emit v half-1
                        # alongside it, and pipeline k half-1 into part 1
                        # (att_part(rt, 1) only needs k tile mt=rt) so the
                        # in-order PE queue always has GEMM filler while
                        # ACT runs the exps.
                        for rt in att_rts:
                            att_part(rt, 0)
                            if 'kv' not in skip:
                                v_gemm(1, rt)
                        if 'kv' not in skip:
                            k_gemm(1, 0)
                        for rt in att_rts:
                            att_part(rt, 1)
                            if 'kv' not in skip and rt < 7:
                                k_gemm(1, rt + 1)

                # ---------- proj + residual -> x2, LN2, MLP ----------
                x2_sb = bigp.tile([128, CH, 512], F32R, tag="xctx")
                h2_sb = bigp.tile([128, CH, 512], DBF, tag="h1h0")
                hm_sb = bigp.tile([128, 32, 512], DBF, tag="ksb")
                out_sb = bigp.tile([128, CH, 512], F32, tag="vsb")
                with (
                    tc.tile_pool(name=f"psln2{it}", bufs=1, space="PSUM") as ps_ln2,
                    tc.tile_pool(name=f"psacc2{it}", bufs=2, space="PSUM") as ps_acc2,
                ):
                    for mt in range(8):
                        wp_sb = wpool.tile([128, CH, 128], DBF, tag="w",
                                           name=f"wp_{it}_{mt}")
                        nc.sync.dma_start(out=wp_sb, in_=wp_t[mt, :, :, :])
                        ps = ps_acc2.tile([128, 512], F32, tag="acc",
                                          name=f"psp_{it}_{mt}")
                        for ci in range(CH):
                            nc.tensor.matmul(ps, wp_sb[:, ci, :],
                                             y_sb[:, ci, :],
                                             start=(ci == 0), stop=(ci == CH - 1))
                        nc.vector.scalar_tensor_tensor(
                            out=x2_sb[:, mt, :], in0=ps, scalar=bp_sb[:, mt:mt + 1],
                            in1=f32(x_own_sb[:, mt, :]),
                            op0=mybir.AluOpType.add, op1=mybir.AluOpType.add)

                    ln_group(x2_sb, 512,
                             lambda ci: h2_sb[:, ci, :], ps_ln2, "ln2",
                             sq_dve=True)

                    mlp_hts = [] if 'mlp' in skip else list(range(32))
                    if 'mlp' in skip:
                        nc.vector.memset(hm_sb, 0.01)
                    for ht in mlp_hts:
                        w1_sb = wpool.tile([128, CH, 128], DBF, tag="w",
                                           name=f"w1_{it}_{ht}")
                        nc.sync.dma_start(out=w1_sb, in_=w1_t[ht, :, :, :])
                        ps = ps_acc2.tile([128, 512], F32, tag="acc",
                                          name=f"psm1_{it}_{ht}")
                        for ci in range(CH):
                            nc.tensor.matmul(ps, w1_sb[:, ci, :],
                                             h2_sb[:, ci, :],
                                             start=(ci == 0), stop=(ci == CH - 1))
                        nc.scalar.activation(hm_sb[:, ht, :], ps,
                                             mybir.ActivationFunctionType.Gelu,
                                             bias=b1_sb[:, ht:ht + 1])

                    for mt in range(8):
                        ps = ps_acc2.tile([128, 512], F32, tag="acc",
                                          name=f"psm2_{it}_{mt}")
                        for qq in range(4):
                            w2_sb = wpool.tile([128, 8, 128], DBF, tag="w",
                                               name=f"w2_{it}_{mt}_{qq}")
                            nc.sync.dma_start(out=w2_sb,
                                              in_=w2_t[mt, :, 8 * qq:8 * qq + 8, :])
                            for hc in range(8):
                                g = 8 * qq + hc
                                nc.tensor.matmul(ps, w2_sb[:, hc, :], hm_sb[:, g, :],
                                                 start=(g == 0), stop=(g == 31))
                        nc.vector.scalar_tensor_tensor(
                            out=out_sb[:, mt, :], in0=ps, scalar=b2_sb[:, mt:mt + 1],
                            in1=f32(x2_sb[:, mt, :]),
                            op0=mybir.AluOpType.add, op1=mybir.AluOpType.add)
                    for mt in range(8):
                        nc.sync.dma_start(out=out_t[mt, :, :], in_=out_sb[:, mt, :])

        if reps > 1:
            with tc.For_i(0, reps, 1):
                body(0)
        else:
            body(0)

    nc.compile()
    _BUILD_CACHE[key] = nc
    return nc


def _tile_w(w, n_chunks, n_mt):
    """[K, M] -> [n_mt, 128, n_chunks, 128] so each lhsT tile is contiguous."""
    return np.ascontiguousarray(
        w.reshape(n_chunks, 128, n_mt, 128).transpose(2, 1, 0, 3))


def _col8(v):
    """[N*128] -> [128, N] per-partition scalar table."""
    n = v.shape[0] // 128
    return np.ascontiguousarray(v.reshape(n, 128).T)


def make_in_maps(x, w_qkv, b_qkv, w_proj, b_proj, ln1_g, ln1_b, ln2_g, ln2_b,
                 w1, b1, w2, b2):
    f = lambda a: np.asarray(a, np.float32)
    x = f(x)
    w_qkv, b_qkv, w_proj, b_proj = f(w_qkv), f(b_qkv), f(w_proj), f(b_proj)
    w1, b1, w2, b2 = f(w1), f(b1), f(w2), f(b2)
    wq, wk, wv = w_qkv[:, 0:C], w_qkv[:, C:2 * C], w_qkv[:, 2 * C:3 * C]
    # fold LN1 gain into qkv weights and LN1 bias into qkv biases; the
    # per-token mean subtraction becomes a rank-1 correction with the
    # negated column sums (cq/ck/cv).  Same for LN2 into w1/b1.
    wq_e = wq * ln1_g[:, None]
    wk_e = wk * ln1_g[:, None]
    wv_e = wv * ln1_g[:, None]
    bq_e = b_qkv[0:C] + wq.T @ ln1_b
    bk_e = b_qkv[C:2 * C] + wk.T @ ln1_b
    bv_e = b_qkv[2 * C:3 * C] + wv.T @ ln1_b
    w1_e = w1 * ln2_g[:, None]
    b1_e = b1 + w1.T @ ln2_b
    common = {
        "wq": _tile_w(wq_e, CH, 8).astype(BF16),
        "wk": _tile_w(wk_e, CH, 8).astype(BF16),
        "wv": np.ascontiguousarray(wv_e.reshape(CH, 128, C)).astype(BF16),
        "wp": _tile_w(w_proj, CH, 8).astype(BF16),
        "w1": _tile_w(w1_e, CH, 32).astype(BF16),
        "w2": _tile_w(w2, 32, 8).astype(BF16),
        "bq": _col8(bq_e), "bk": _col8(bk_e),
        "bp": _col8(b_proj + w_proj.T @ bv_e), "b1": _col8(b1_e), "b2": _col8(b2),
    }
    in_maps = []
    for c in range(NC):
        seq = c // 4
        j = c % 4
        xf = np.ascontiguousarray(x[seq].T)          # [C, T] feature-major
        own = np.arange(j, T, 4)                     # stride-4 interleave
        x_own = xf[:, own]
        # masks for the boundary k-tiles of each q-tile: q-tile qi covers
        # positions 512*qi + 4*qq + j; its k-tiles 4*qi..4*qi+3 are partial.
        kk = np.arange(128)
        qq = np.arange(QT)
        masks = np.zeros((16, 128, QT), np.float32)
        for qi in range(4):
            qpos = 512 * qi + 4 * qq + j
            for m_ in range(4):
                kt = 4 * qi + m_
                masks[4 * qi + m_] = qpos[None, :] >= (128 * kt + kk[:, None])
        m = {
            "x_ctx": np.ascontiguousarray(xf.reshape(CH, 128, T)),
            "x_own": np.ascontiguousarray(x_own.reshape(CH, 128, 512)),
            "mask": np.ascontiguousarray(masks.transpose(1, 0, 2)).astype(BF16),
        }
        m.update(common)
        in_maps.append(m)
    return in_maps


def assemble_output(results):
    out = np.zeros((B, T, C), np.float32)
    for c in range(NC):
        seq = c // 4
        j = c % 4
        yf = results[c]["out"].reshape(C, 512)       # feature-major [C, 512]
        out[seq, j::4, :] = yf.T
    return out


def kernel(**inputs):
    nc = build_kernel(reps=1)
    in_maps = make_in_maps(**inputs)
    res = run_bass_kernel_spmd(nc, in_maps, list(range(NC)))
    return assemble_output(res.results)



# revision 42
# speedup vs baseline: 4.3776x; 4.3776x over previous
"""Trainium2 Bass kernel for a dense transformer block (B=2, T=2048, C=1024, 16 heads).

Strategy (zero-collective, uniform SPMD over 8 cores):
  - cores 0-3 handle batch 0, cores 4-7 batch 1 (4 cores per sequence)
  - core with j = core%4 owns the stride-4 interleaved tokens {4u+j}: its
    q-tile qi (128 tokens covering positions [512*qi, 512*qi+512)) attends
    exactly k-tiles [0, 4*(qi+1)) with only the last 4 masked, so every
    core runs an identical, exactly-causal instruction stream (4+8+12+16
    = 40 k-tile units vs 34 true causal); masks are per-core data.
  - every core recomputes LN1 + k/v for its full 2048-token sequence,
    which removes all cross-core communication (collectives measured
    ~430-520us per op here -- far too slow to shard k/v).
  - LayerNorm subtracts the mean directly (DVE/Pool + gpsimd
    partition_broadcast; no rank-1 correction matmuls); the v bias is
    folded into the proj bias on the host (softmax rows sum to 1).
  - half-1 k/v GEMM emission is interleaved with attention part 0 so the
    in-order PE queue has matmul work while ACT runs the softmax exps.
  - attention (q/k/v/exp), proj and mlp2 run bf16; LN stats and mlp1 use
    float32r on f32 data (full-rate on the PE at N>=256).

kernel(**inputs) takes the full unsharded inputs and returns the full
[2, 2048, 1024] output.
"""
import numpy as np
import ml_dtypes

import concourse.bass as bass
import concourse.tile as tile
from concourse import bacc, mybir
from concourse.bass_utils import run_bass_kernel_spmd

BF16 = ml_dtypes.bfloat16
F32 = mybir.dt.float32
F32R = mybir.dt.float32r
DBF = mybir.dt.bfloat16

C = 1024          # embed dim
T = 2048          # seq len
B = 2
H = 16            # heads
D = 64            # head dim
HID = 4096
NC = 8            # cores
CH = C // 128     # 8 channel chunks
QT = 128          # query tile width (stride-4 interleaved tokens)
LN_EPS = 1e-5
ATT_SCALE = 1.0 / 8.0   # 1/sqrt(64)

_BUILD_CACHE = {}


def r32(ap):
    """View an f32 access pattern as float32r for full-rate matmuls."""
    return ap.bitcast(F32R)


def f32(ap):
    """View a float32r access pattern as plain f32 for vector-engine ops."""
    return ap.bitcast(F32)


def build_kernel(reps=1, skip=()):
    key = (reps, tuple(skip))
    if key in _BUILD_CACHE:
        return _BUILD_CACHE[key]
    nc = bacc.Bacc("TRN2", target_bir_lowering=False, debug=False, num_devices=NC)

    # ---- I/O ----
    x_ctx_t = nc.dram_tensor("x_ctx", [CH, 128, T], F32R, kind="ExternalInput")
    x_own_t = nc.dram_tensor("x_own", [CH, 128, 512], F32R, kind="ExternalInput")
    wq_t = nc.dram_tensor("wq", [8, 128, CH, 128], DBF, kind="ExternalInput")
    wk_t = nc.dram_tensor("wk", [8, 128, CH, 128], DBF, kind="ExternalInput")
    wv_t = nc.dram_tensor("wv", [CH, 128, C], DBF, kind="ExternalInput")
    wp_t = nc.dram_tensor("wp", [8, 128, CH, 128], DBF, kind="ExternalInput")
    w1_t = nc.dram_tensor("w1", [32, 128, CH, 128], DBF, kind="ExternalInput")
    w2_t = nc.dram_tensor("w2", [8, 128, 32, 128], DBF, kind="ExternalInput")
    bq_t = nc.dram_tensor("bq", [128, 8], F32, kind="ExternalInput")
    bk_t = nc.dram_tensor("bk", [128, 8], F32, kind="ExternalInput")
    bp_t = nc.dram_tensor("bp", [128, 8], F32, kind="ExternalInput")
    b1_t = nc.dram_tensor("b1", [128, 32], F32, kind="ExternalInput")
    b2_t = nc.dram_tensor("b2", [128, 8], F32, kind="ExternalInput")
    mask_t = nc.dram_tensor("mask", [128, 16, QT], DBF, kind="ExternalInput")
    out_t = nc.dram_tensor("out", [CH, 128, 512], F32, kind="ExternalOutput")

    with tile.TileContext(nc) as tc, nc.allow_low_precision(
            reason="float32r operands feeding full-rate matmuls"):
        def body(it):
            with (
                tc.tile_pool(name=f"const{it}", bufs=1) as const,
                tc.tile_pool(name=f"bigp{it}", bufs=1) as bigp,
                tc.tile_pool(name=f"wpool{it}", bufs=2) as wpool,
                tc.tile_pool(name=f"stat{it}", bufs=1) as stat,
                tc.tile_pool(name=f"tmp{it}", bufs=2) as tmp,
            ):
                ones_f = const.tile([128, 128], F32)
                nc.vector.memset(ones_f, 1.0)
                ones_col = const.tile([128, 1], F32R)
                nc.vector.tensor_copy(ones_col, ones_f[:, 0:1])
                ones_row = const.tile([1, 128], F32R)
                nc.vector.tensor_copy(ones_row, ones_f[0:1, :])
                eps_c = const.tile([1, 1], F32)
                nc.vector.memset(eps_c, LN_EPS)

                def cload(name, t, shape, dtype=F32):
                    s = const.tile(shape, dtype, name=name)
                    nc.sync.dma_start(out=s, in_=t[tuple(slice(None) for _ in shape)])
                    return s

                x_own_sb = bigp.tile([128, CH, 512], F32R, tag="xown")
                for ci in range(CH):
                    nc.sync.dma_start(out=x_own_sb[:, ci, :], in_=x_own_t[ci, :, :])

                bq_sb = cload("bqs", bq_t, [128, 8])
                bk_sb = cload("bks", bk_t, [128, 8])

                def ln_group(x_ap, w, out_ap_fn, ps_ln, gi, sq_dve=False):
                    """LayerNorm of one <=512-token group: writes
                    out = (x - mu) * rstd (per token).  LN gain/bias are
                    folded into the following weights/biases on the host."""
                    ps_s = ps_ln.tile([1, 512], F32, tag="s", bufs=2, name=f"pss_{it}_{gi}")
                    ps_q = ps_ln.tile([1, 512], F32, tag="q", bufs=2, name=f"psq_{it}_{gi}")
                    for ci in range(CH):
                        nc.tensor.matmul(
                            ps_s[:, :w], ones_col, x_ap[:, ci, :],
                            start=(ci == 0), stop=(ci == CH - 1))
                    for ci in range(CH):
                        sq = tmp.tile([128, 512], F32R, tag="sq", bufs=1,
                                      name=f"sq_{it}_{gi}_{ci}")
                        sq_eng = nc.vector if sq_dve else nc.gpsimd
                        sq_eng.tensor_mul(sq[:, :w], f32(x_ap[:, ci, :]),
                                          f32(x_ap[:, ci, :]))
                        nc.tensor.matmul(
                            ps_q[:, :w], ones_col, sq[:, :w],
                            start=(ci == 0), stop=(ci == CH - 1))
                    mu = stat.tile([1, 512], F32, tag="mu", name=f"mu_{it}_{gi}")
                    nc.vector.tensor_scalar_mul(mu[:, :w], ps_s[:, :w], 1.0 / C)
                    ex2 = stat.tile([1, 512], F32, tag="ex2", name=f"ex2_{it}_{gi}")
                    nc.vector.tensor_scalar_mul(ex2[:, :w], ps_q[:, :w], 1.0 / C)
                    var = stat.tile([1, 512], F32, tag="var", name=f"var_{it}_{gi}")
                    nc.scalar.activation(var[:, :w], mu[:, :w],
                                         mybir.ActivationFunctionType.Square)
                    nc.vector.tensor_sub(var[:, :w], ex2[:, :w], var[:, :w])
                    nc.scalar.activation(var[:, :w], var[:, :w],
                                         mybir.ActivationFunctionType.Sqrt,
                                         bias=eps_c[:, :])
                    rstd = stat.tile([1, 512], F32, tag="ex2", name=f"rstd_{it}_{gi}")
                    nc.vector.reciprocal(rstd[:, :w], var[:, :w])
                    mub = tmp.tile([128, 512], F32, tag="mub", bufs=2,
                                   name=f"mub_{it}_{gi}")
                    nc.gpsimd.partition_broadcast(mub[:, :w], mu[:, :w])
                    rsb = tmp.tile([128, 512], F32, tag="rsb", bufs=2,
                                   name=f"rsb_{it}_{gi}")
                    nc.gpsimd.partition_broadcast(rsb[:, :w], rstd[:, :w])
                    for ci in range(CH):
                        xc = tmp.tile([128, 512], F32, tag="xc", bufs=2,
                                      name=f"xc_{it}_{gi}_{ci}")
                        nc.gpsimd.tensor_sub(xc[:, :w], f32(x_ap[:, ci, :]),
                                             mub[:, :w])
                        nc.vector.tensor_mul(out_ap_fn(ci), xc[:, :w],
                                             rsb[:, :w])

                h1o = bigp.tile([128, CH, 512], DBF, tag="h1o")
                q_sb = bigp.tile([128, 8, 512], DBF, tag="qsb")
                k_sb = bigp.tile([128, 8, T], DBF, tag="ksb")
                v_sb = bigp.tile([128, 16, H, D + 1], DBF, tag="vsb")
                nc.vector.memset(v_sb[:, :, :, D:D + 1], 1.0)
                if 'kv' in skip:
                    nc.vector.memset(k_sb, 0.01)
                    nc.vector.memset(v_sb[:, :, :, 0:D], 0.01)

                y_sb = bigp.tile([128, 8, 512], DBF, tag="h1o")

                with tc.tile_pool(name=f"psacc{it}", bufs=2, space="PSUM") as ps_acc:
                    wv_sb = bigp.tile([128, CH, C], DBF, tag="wv")
                    h1hs = {}

                    def k_gemm(half, mt):
                        t0 = 1024 * half
                        h1h = h1hs[half]
                        wk_sb = wpool.tile([128, CH, 128], DBF, tag="w",
                                           name=f"wk_{it}_{half}_{mt}")
                        nc.sync.dma_start(out=wk_sb, in_=wk_t[mt, :, :, :])
                        for g in range(2):
                            g0 = t0 + 512 * g
                            ps = ps_acc.tile([128, 512], F32, tag="acc",
                                             name=f"psk_{it}_{half}_{mt}_{g}")
                            for ci in range(CH):
                                nc.tensor.matmul(
                                    ps, wk_sb[:, ci, :],
                                    h1h[:, ci, 512 * g:512 * g + 512],
                                    start=(ci == 0), stop=(ci == CH - 1))
                            nc.vector.tensor_scalar_add(
                                k_sb[:, mt, g0:g0 + 512], ps,
                                bk_sb[:, mt:mt + 1])

                    def v_gemm(half, tt):
                        # v bias is folded into the proj bias on the host
                        # (softmax rows sum to one, so y = AV/den + bv exactly)
                        h1h = h1hs[half]
                        gtt = 8 * half + tt
                        for vh in range(2):
                            ps = ps_acc.tile([128, 512], F32, tag="acc",
                                             name=f"psv_{it}_{half}_{tt}_{vh}")
                            for ci in range(CH):
                                nc.tensor.matmul(
                                    ps, h1h[:, ci, 128 * tt:128 * tt + 128],
                                    wv_sb[:, ci, 512 * vh:512 * vh + 512],
                                    start=(ci == 0), stop=(ci == CH - 1))
                            nc.vector.tensor_copy(
                                v_sb[:, gtt, 8 * vh:8 * vh + 8, 0:D],
                                ps.rearrange("p (h d) -> p h d", h=8))

                    with tc.tile_pool(name=f"psln{it}", bufs=1,
                                      space="PSUM") as ps_ln:
                        # ---- LN1 over own tokens -> h1o, then q ----
                        # (ctx group c00's stats are emitted between the two
                        # so PE has matmul work during the own-normalize tail)
                        ln_group(x_own_sb, 512,
                                 lambda ci: h1o[:, ci, :], ps_ln, "own",
                                 sq_dve=True)
                        h1hs[0] = bigp.tile([128, CH, 1024], DBF,
                                            tag="h1h0", name=f"h1h_{it}_0")
                        xg0 = bigp.tile([128, CH, 512], F32R, tag="xctx",
                                        name=f"xg_{it}_0_0")
                        for ci in range(CH):
                            nc.sync.dma_start(out=xg0[:, ci, :],
                                              in_=x_ctx_t[ci, :, 0:512])
                        ln_group(xg0, 512,
                                 lambda ci: h1hs[0][:, ci, 0:512],
                                 ps_ln, "c00")
                        for mt in range(8):
                            wq_sb = wpool.tile([128, CH, 128], DBF, tag="w",
                                               name=f"wq_{it}_{mt}")
                            nc.sync.dma_start(out=wq_sb, in_=wq_t[mt, :, :, :])
                            ps = ps_acc.tile([128, 512], F32, tag="acc",
                                             name=f"psq2_{it}_{mt}")
                            for ci in range(CH):
                                nc.tensor.matmul(ps, wq_sb[:, ci, :],
                                                 h1o[:, ci, :],
                                                 start=(ci == 0),
                                                 stop=(ci == CH - 1))
                            nc.vector.tensor_scalar_add(
                                q_sb[:, mt, :], ps, bq_sb[:, mt:mt + 1])

                        for ci in range(CH):
                            nc.sync.dma_start(out=wv_sb[:, ci, :],
                                              in_=wv_t[ci, :, :])
                        bp_sb = cload("bps", bp_t, [128, 8])
                        b1_sb = cload("b1s", b1_t, [128, 32])
                        b2_sb = cload("b2s", b2_t, [128, 8])
                        mask_sb = cload("masks", mask_t, [128, 16, QT], DBF)

                        # ---- LN1 of remaining ctx groups, all before the
                        # half-0 k/v GEMMs: the GEMMs then provide PE filler
                        # while the later groups' normalize chains run, and
                        # the half-1 x_ctx stages load early enough for the
                        # interleaved half-1 k/v during attention part 0 ----
                        h1hs[1] = bigp.tile([128, CH, 1024], DBF, tag="h1h1",
                                            name=f"h1h_{it}_1")
                        for half, g in ((0, 1), (1, 0), (1, 1)):
                            g0 = 1024 * half + 512 * g
                            xg = bigp.tile([128, CH, 512], F32R, tag="xctx",
                                           name=f"xg_{it}_{half}_{g}")
                            for ci in range(CH):
                                nc.sync.dma_start(
                                    out=xg[:, ci, :],
                                    in_=x_ctx_t[ci, :, g0:g0 + 512])
                            ln_group(
                                xg, 512,
                                lambda ci, half=half, g=g: h1hs[half][
                                    :, ci, 512 * g:512 * g + 512],
                                ps_ln, f"c{half}{g}")
                        if 'kv' not in skip:
                            for mt in range(8):
                                k_gemm(0, mt)
                            for tt in range(8):
                                v_gemm(0, tt)

                    # ---------- attention (stride-4 interleaved queries),
                    # half-1 k/v GEMMs emission-interleaved with part 0 so PE
                    # has matmul work while ACT runs softmax exps ----------
                    # q-tile qi = positions {512*qi + 4u + j}; it attends
                    # exactly k-tiles [0, 4*(qi+1)), the last 4 masked.
                    with (
                        tc.tile_pool(name=f"psscr{it}", bufs=2,
                                     space="PSUM") as ps_scr,
                        tc.tile_pool(name=f"psy{it}", bufs=1,
                                     space="PSUM") as ps_y,
                        tc.tile_pool(name=f"esp{it}", bufs=2) as espool,
                    ):
                        def att_part(rt, part):
                            hh = [2 * rt, 2 * rt + 1]
                            psYs = [ps_y.tile([D + 1, 256], F32, tag=f"y{u}",
                                              name=f"psY_{it}_{rt}_{part}_{u}")
                                    for u in range(2)]
                            for qq_ in range(2):
                                qi = 2 * part + qq_
                                q0 = QT * qi
                                nkt = 4 * (qi + 1)
                                ess = [espool.tile([128, 16, QT], DBF,
                                                   tag=f"es{u}",
                                                   name=f"es_{it}_{rt}_{qi}_{u}")
                                       for u in range(2)]
                                for kb in range(qi + 1):
                                    pss = [ps_scr.tile(
                                        [128, 4, QT], F32, tag=f"scr{u}",
                                        name=f"psS_{it}_{rt}_{qi}_{kb}_{u}")
                                        for u in range(2)]
                                    for i in range(4):
                                        kt = 4 * kb + i
                                        for u in range(2):
                                            po = 64 * u
                                            nc.tensor.matmul(
                                                pss[u][:, i, :],
                                                k_sb[po:po + 64, rt,
                                                     128 * kt:128 * kt + 128],
                                                q_sb[po:po + 64, rt,
                                                     q0:q0 + QT],
                                                start=True, stop=True)
                                    for u in range(2):
                                        nc.scalar.activation(
                                            ess[u][:, 4 * kb:4 * kb + 4, :],
                                            pss[u],
                                            mybir.ActivationFunctionType.Exp,
                                            scale=ATT_SCALE)
                                        if kb == qi:   # boundary k-tiles
                                            nc.vector.tensor_mul(
                                                ess[u][:, 4 * kb:4 * kb + 4, :],
                                                ess[u][:, 4 * kb:4 * kb + 4, :],
                                                mask_sb[:, 4 * qi:4 * qi + 4, :])
                                    for u in range(2):
                                        for i in range(4):
                                            kt = 4 * kb + i
                                            nc.tensor.matmul(
                                                psYs[u][:, 128 * qq_:
                                                        128 * qq_ + 128],
                                                v_sb[:, kt, hh[u], :],
                                                ess[u][:, kt, :],
                                                start=(kt == 0),
                                                stop=(kt == nkt - 1),
                                                skip_group_check=True)
                            for u in range(2):
                                po = 64 * u
                                psY = psYs[u]
                                rd = stat.tile([1, 256], F32, tag="mu",
                                               name=f"rd_{it}_{rt}_{part}_{u}")
                                nc.vector.reciprocal(rd, psY[D:D + 1, :])
                                rdb = tmp.tile([64, 256], F32, tag="rdb",
                                               bufs=2,
                                               name=f"rdb_{it}_{rt}_{part}_{u}")
                                nc.gpsimd.partition_broadcast(rdb, rd)
                                nc.vector.tensor_mul(
                                    y_sb[po:po + 64, rt,
                                         256 * part:256 * part + 256],
                                    psY[0:D, :], rdb)

                        att_rts = [] if 'att' in skip else list(range(8))
                        if 'att' in skip:
                            nc.vector.tensor_copy(y_sb.rearrange('p a b -> p (a b)'), ones_f[:, 0:1].broadcast_to((128, CH * 512)))
                        # part 0 needs only half-0 k/v; emit v half-1
                        # alongside it, and pipeline k half-1 into part 1
                        # (att_part(rt, 1) only needs k tile mt=rt) so the
                        # in-order PE queue always has GEMM filler while
                        # ACT runs the exps.
                        for rt in att_rts:
                            att_part(rt, 0)
                            if 'kv' not in skip:
                                v_gemm(1, rt)
                        if 'kv' not in skip:
                            k_gemm(1, 0)
                        for rt in att_rts:
                            att_part(rt, 1)
                            if 'kv' not in skip and rt < 7:
                                k_gemm(1, rt + 1)

                # ---------- proj + residual -> x2, LN2, MLP ----------
                x2_sb = bigp.tile([128, CH, 512], F32R, tag="xctx")
                h2_sb = bigp.tile([128, CH, 512], DBF, tag="h1h0")
                hm_sb = bigp.tile([128, 32, 512], DBF, tag="ksb")
                out_sb = bigp.tile([128, CH, 512], F32, tag="vsb")
                with (
                    tc.tile_pool(name=f"psln2{it}", bufs=1, space="PSUM") as ps_ln2,
                    tc.tile_pool(name=f"psacc2{it}", bufs=2, space="PSUM") as ps_acc2,
                ):
                    for mt in range(8):
                        wp_sb = wpool.tile([128, CH, 128], DBF, tag="w",
                                           name=f"wp_{it}_{mt}")
                        nc.sync.dma_start(out=wp_sb, in_=wp_t[mt, :, :, :])
                        ps = ps_acc2.tile([128, 512], F32, tag="acc",
                                          name=f"psp_{it}_{mt}")
                        for ci in range(CH):
                            nc.tensor.matmul(ps, wp_sb[:, ci, :],
                                             y_sb[:, ci, :],
                                             start=(ci == 0), stop=(ci == CH - 1))
                        nc.vector.scalar_tensor_tensor(
                            out=x2_sb[:, mt, :], in0=ps, scalar=bp_sb[:, mt:mt + 1],
                            in1=f32(x_own_sb[:, mt, :]),
                            op0=mybir.AluOpType.add, op1=mybir.AluOpType.add)

                    ln_group(x2_sb, 512,
                             lambda ci: h2_sb[:, ci, :], ps_ln2, "ln2",
                             sq_dve=True)

                    mlp_hts = [] if 'mlp' in skip else list(range(32))
                    if 'mlp' in skip:
                        nc.vector.memset(hm_sb, 0.01)
                    for ht in mlp_hts:
                        w1_sb = wpool.tile([128, CH, 128], DBF, tag="w",
                                           name=f"w1_{it}_{ht}")
                        nc.sync.dma_start(out=w1_sb, in_=w1_t[ht, :, :, :])
                        ps = ps_acc2.tile([128, 512], F32, tag="acc",
                                          name=f"psm1_{it}_{ht}")
                        for ci in range(CH):
                            nc.tensor.matmul(ps, w1_sb[:, ci, :],
                                             h2_sb[:, ci, :],
                                             start=(ci == 0), stop=(ci == CH - 1))
                        nc.scalar.activation(hm_sb[:, ht, :], ps,
                                             mybir.ActivationFunctionType.Gelu,
                                             bias=b1_sb[:, ht:ht + 1])

                    for mt in range(8):
                        ps = ps_acc2.tile([128, 512], F32, tag="acc",
                                          name=f"psm2_{it}_{mt}")
                        for qq in range(4):
                            w2_sb = wpool.tile([128, 8, 128], DBF, tag="w",
                                               name=f"w2_{it}_{mt}_{qq}")
                            nc.sync.dma_start(out=w2_sb,
                                              in_=w2_t[mt, :, 8 * qq:8 * qq + 8, :])
                            for hc in range(8):
                                g = 8 * qq + hc
                                nc.tensor.matmul(ps, w2_sb[:, hc, :], hm_sb[:, g, :],
                                                 start=(g == 0), stop=(g == 31))
                        nc.vector.scalar_tensor_tensor(
                            out=out_sb[:, mt, :], in0=ps, scalar=b2_sb[:, mt:mt + 1],
                            in1=f32(x2_sb[:, mt, :]),
                            op0=mybir.AluOpType.add, op1=mybir.AluOpType.add)
                    for mt in range(8):
                        nc.sync.dma_start(out=out_t[mt, :, :], in_=out_sb[:, mt, :])

        if reps > 1:
            with tc.For_i(0, reps, 1):
                body(0)
        else:
            body(0)

    nc.compile()
    _BUILD_CACHE[key] = nc
    return nc


def _tile_w(w, n_chunks, n_mt):
    """[K, M] -> [n_mt, 128, n_chunks, 128] so each lhsT tile is contiguous."""
    return np.ascontiguousarray(
        w.reshape(n_chunks, 128, n_mt, 128).transpose(2, 1, 0, 3))


def _col8(v):
    """[N*128] -> [128, N] per-partition scalar table."""
    n = v.shape[0] // 128
    return np.ascontiguousarray(v.reshape(n, 128).T)


def make_in_maps(x, w_qkv, b_qkv, w_proj, b_proj, ln1_g, ln1_b, ln2_g, ln2_b,
                 w1, b1, w2, b2):
    f = lambda a: np.asarray(a, np.float32)
    x = f(x)
    w_qkv, b_qkv, w_proj, b_proj = f(w_qkv), f(b_qkv), f(w_proj), f(b_proj)
    w1, b1, w2, b2 = f(w1), f(b1), f(w2), f(b2)
    wq, wk, wv = w_qkv[:, 0:C], w_qkv[:, C:2 * C], w_qkv[:, 2 * C:3 * C]
    # fold LN1 gain into qkv weights and LN1 bias into qkv biases; the
    # per-token mean subtraction becomes a rank-1 correction with the
    # negated column sums (cq/ck/cv).  Same for LN2 into w1/b1.
    wq_e = wq * ln1_g[:, None]
    wk_e = wk * ln1_g[:, None]
    wv_e = wv * ln1_g[:, None]
    bq_e = b_qkv[0:C] + wq.T @ ln1_b
    bk_e = b_qkv[C:2 * C] + wk.T @ ln1_b
    bv_e = b_qkv[2 * C:3 * C] + wv.T @ ln1_b
    w1_e = w1 * ln2_g[:, None]
    b1_e = b1 + w1.T @ ln2_b
    common = {
        "wq": _tile_w(wq_e, CH, 8).astype(BF16),
        "wk": _tile_w(wk_e, CH, 8).astype(BF16),
        "wv": np.ascontiguousarray(wv_e.reshape(CH, 128, C)).astype(BF16),
        "wp": _tile_w(w_proj, CH, 8).astype(BF16),
        "w1": _tile_w(w1_e, CH, 32).astype(BF16),
        "w2": _tile_w(w2, 32, 8).astype(BF16),
        "bq": _col8(bq_e), "bk": _col8(bk_e),
        "bp": _col8(b_proj + w_proj.T @ bv_e), "b1": _col8(b1_e), "b2": _col8(b2),
    }
    in_maps = []
    for c in range(NC):
        seq = c // 4
        j = c % 4
        xf = np.ascontiguousarray(x[seq].T)          # [C, T] feature-major
        own = np.arange(j, T, 4)                     # stride-4 interleave
        x_own = xf[:, own]
        # masks for the boundary k-tiles of each q-tile: q-tile qi covers
        # positions 512*qi + 4*qq + j; its k-tiles 4*qi..4*qi+3 are partial.
        kk = np.arange(128)
        qq = np.arange(QT)
        masks = np.zeros((16, 128, QT), np.float32)
        for qi in range(4):
            qpos = 512 * qi + 4 * qq + j
            for m_ in range(4):
                kt = 4 * qi + m_
                masks[4 * qi + m_] = qpos[None, :] >= (128 * kt + kk[:, None])
        m = {
            "x_ctx": np.ascontiguousarray(xf.reshape(CH, 128, T)),
            "x_own": np.ascontiguousarray(x_own.reshape(CH, 128, 512)),
            "mask": np.ascontiguousarray(masks.transpose(1, 0, 2)).astype(BF16),
        }
        m.update(common)
        in_maps.append(m)
    return in_maps


def assemble_output(results):
    out = np.zeros((B, T, C), np.float32)
    for c in range(NC):
        seq = c // 4
        j = c % 4
        yf = results[c]["out"].reshape(C, 512)       # feature-major [C, 512]
        out[seq, j::4, :] = yf.T
    return out


def kernel(**inputs):
    nc = build_kernel(reps=1)
    in_maps = make_in_maps(**inputs)
    res = run_bass_kernel_spmd(nc, in_maps, list(range(NC)))
    return assemble_output(res.results)

